# revision 51
# baseline (speedup 1.0000x reference)
"""Trainium2 Bass kernel for nn_Decoder_16054587752897.

Decoder block: banded additive (Bahdanau) attention + LN + FFN + LN +
3x (nearest-upsample-2x + conv1d k=7 + relu) + conv1d k=11 + sigmoid.

Sharding: pure data parallel - batch N=8, one batch element per NeuronCore.

Key optimizations over the v1 kernel (112us):
 - PE HAM warmup: dummy matmul stream during the DMA prologue flips the
   clock gate to 8/8 (2.4 GHz) before real matmuls start; small heater
   bursts in long PE-idle phases prevent re-throttle.
 - Parallel prologue: input + 3 weight blobs on 4 different engine queues.
 - Softmax normalization dropped entirely: LayerNorm is invariant to a
   per-position positive scale, so the AV matmul computes
   u_i = sum_o e[i,o] x_j + S_i x_i (S folded into the band center
   column) and LN0(u) == LN0(v + x) to ~1e-7.
 - LN rstd via exp(-0.5*ln(var+eps)) so the whole kernel uses only two
   ACT table sets (exp_and_others -> natural_log_exp_and_others), one
   mid-kernel switch, both triggered early by dummy ops off the chain.
 - Final sigmoid via exp(-ln(1+exp(-x))) on the same table set, computed
   on an [8, 512] layout (output conv written to 8 partitions via
   per-chunk one-hot lhsT columns) instead of [1, 4096] on one lane.
 - conv2/conv3 tap-pair packing (pack2): 2 accumulating matmuls instead
   of 4 per output tile, fed by double-written (shifted) activations.
 - h3rep tap replicas via 3 parallel-queue DMAs instead of serial.
"""

import os
import sys

for _p in ("/opt/trn_rl_repo",):
    if _p not in sys.path:
        sys.path.insert(0, _p)

import math
import numpy as np
from contextlib import ExitStack

import concourse.bass as bass
import concourse.bacc as bacc
import concourse.mybir as mybir
import concourse.tile as tile
from concourse.bass_utils import run_bass_kernel_spmd

F32 = mybir.dt.float32
BF16 = mybir.dt.bfloat16
AF = mybir.ActivationFunctionType
ALU = mybir.AluOpType
AX = mybir.AxisListType

L = 512
C = 96
EPS_LN = 1e-5
LN96 = math.log(96.0)

N_HEAT_PROLOGUE = 12
N_HEAT_SOFTMAX = 6


# ----------------------------------------------------------------------------
# host-side constant prep (weight-only transforms)
# ----------------------------------------------------------------------------

def _host_prep(inp):
    f = lambda k: np.ascontiguousarray(np.asarray(inp[k], np.float32))
    p = {}
    p['Wt'] = f('Wt')                       # [96, 32] lhsT for q
    p['Wx'] = f('Wx')                       # [96, 32] lhsT for k
    Wa = f('Wa')[:, 0]
    blockWa4 = np.zeros((128, 4), np.float32)
    for c in range(4):
        blockWa4[32 * c:32 * c + 32, c] = Wa
    p['blockWa4'] = blockWa4
    p['bh4col'] = np.tile(f('bh'), 4).reshape(128, 1)
    il = np.arange(128)[:, None, None]
    cc = np.arange(4)[None, :, None]
    oo = np.arange(64)[None, None, :]
    jj = cc * 128 + il + oo - 32
    p['bmask16'] = ((jj >= 0) & (jj < L)).astype(np.float32).reshape(128, 256)
    p['identity'] = np.eye(128, dtype=np.float32)
    p['identb'] = np.eye(128, dtype=np.float32)
    p['onesb'] = np.ones((96, 1), np.float32)
    p['one1b'] = np.ones((1, 96), np.float32)
    p['g0row'] = f('ln0_g').reshape(1, 96)
    p['g1row'] = f('ln1_g').reshape(1, 96)
    p['b0col'] = f('ln0_b').reshape(96, 1)
    p['b1col'] = f('ln1_b').reshape(96, 1)
    p['w0T'] = np.ascontiguousarray(f('ff_w0').T)                # [96, 384]
    p['fb0'] = np.ascontiguousarray(f('ff_b0').reshape(3, 128).T)  # [128, 3]
    # w1T [128, 3*96]: cols s*96+c = ff_w1[c, s*128+h]
    w1 = f('ff_w1')                                              # [96, 384]
    w1T = np.zeros((128, 288), np.float32)
    for s in range(3):
        w1T[:, s * 96:(s + 1) * 96] = w1[:, s * 128:(s + 1) * 128].T
    p['w1T'] = w1T
    p['fb1col'] = f('ff_b1').reshape(96, 1)

    def eo(w):
        # w: [co, ci, 7] -> even/odd tap-summed lhsT banks [ci, 4*co]
        We = np.stack([w[:, :, 0], w[:, :, 1] + w[:, :, 2],
                       w[:, :, 3] + w[:, :, 4], w[:, :, 5] + w[:, :, 6]])
        Wo = np.stack([w[:, :, 0] + w[:, :, 1], w[:, :, 2] + w[:, :, 3],
                       w[:, :, 4] + w[:, :, 5], w[:, :, 6]])
        co, ci = w.shape[0], w.shape[1]
        pack = lambda Ws: np.ascontiguousarray(
            Ws.transpose(2, 0, 1).reshape(ci, 4 * co))
        return pack(We), pack(Wo)

    p['W1e'], p['W1o'] = eo(f('up_w0'))   # [96, 256]
    W2e, W2o = eo(f('up_w1'))             # [64, 192]
    W3e, W3o = eo(f('up_w2'))             # [48, 128]

    def pack2(W, ci, co):
        # W [ci, 4*co] tap-major -> [2*ci, 2*co]: rows tau*ci+c_i,
        # group g covers taps (2g, 2g+1)
        out = np.zeros((2 * ci, 2 * co), np.float32)
        for g in range(2):
            for tau in range(2):
                t = 2 * g + tau
                out[tau * ci:(tau + 1) * ci, g * co:(g + 1) * co] = \
                    W[:, t * co:(t + 1) * co]
        return out
    p['W2e2'] = pack2(W2e, 64, 48)   # [128, 96]
    p['W2o2'] = pack2(W2o, 64, 48)

    def pack2g(W, ci, co):
        # like pack2, but the tau=1 row block sits at partition 64 (engine
        # writes must start 32-aligned, so the shifted h2 copy lives at
        # rows 64:64+ci with zero-weight gap rows in between)
        out = np.zeros((64 + ci, 2 * co), np.float32)
        for g in range(2):
            for tau in range(2):
                t = 2 * g + tau
                out[tau * 64:tau * 64 + ci, g * co:(g + 1) * co] = \
                    W[:, t * co:(t + 1) * co]
        return out
    p['W3e2'] = pack2g(W3e, 48, 32)   # [112, 64]
    p['W3o2'] = pack2g(W3o, 48, 32)
    p['cb1'] = f('up_b0').reshape(64, 1)
    p['cb2'] = f('up_b1').reshape(48, 1)
    p['cb3'] = f('up_b2').reshape(32, 1)
    ow = f('out_w')[0]                    # (32, 11)
    # Wog8 [128, 192]: block m=(g*8+k) is [128, 8] with only col k nonzero
    # = tap-group-g column; accumulating all 24 into one [8, 512] PSUM bank
    # puts output chunk k on partition k.
    Wog8 = np.zeros((128, 192), np.float32)
    for g in range(3):
        col = np.zeros(128, np.float32)
        for r in range(4):
            t = 4 * g + r
            if t < 11:
                col[32 * r:32 * r + 32] = ow[:, t]
        for k in range(8):
            Wog8[:, (g * 8 + k) * 8 + k] = col
    p['Wog8'] = Wog8
    p['obh8'] = np.full((8, 1), f('out_b')[0] / 2.0, np.float32)
    p['onesc'] = np.ones((97, 1), np.float32)

    packed = {}
    for blob, names in (('wf32', F32_PACK), ('wb16a', B16A_PACK),
                        ('wb16c', B16C_PACK)):
        width = sum(p[n].shape[1] for n in names)
        buf = np.zeros((128, width), np.float32)
        col = 0
        for n in names:
            a = p[n]
            buf[:a.shape[0], col:col + a.shape[1]] = a
            col += a.shape[1]
        packed[blob] = buf
    shapes = {n: p[n].shape for n in F32_PACK + B16A_PACK + B16C_PACK}
    packed['shapes'] = shapes
    return packed


F32_PACK = ('identity', 'bh4col', 'b0col', 'b1col', 'fb0', 'fb1col',
            'cb1', 'cb2', 'cb3', 'obh8')
B16A_PACK = ('Wt', 'Wx', 'blockWa4', 'bmask16', 'one1b', 'onesc',
             'g0row', 'g1row', 'identb')
B16C_PACK = ('w0T', 'w1T', 'W1e', 'W1o', 'W2e2', 'W2o2', 'W3e2', 'W3o2',
             'Wog8')


# ----------------------------------------------------------------------------
# device kernel build
# ----------------------------------------------------------------------------

def _bcast_free(ap_full, offset_ap, counts):
    """Custom AP on the same tensor: dims [[pstep, 128]] + counts pairs."""
    pstep = ap_full.ap[0][0]
    return bass.AP(ap_full.tensor, offset_ap.offset,
                   [[pstep, ap_full.ap[0][1]]] + list(counts))


def _build(nc, tc, t_in, t_out, tp):
    x_ap = t_in.ap()          # [96, 512]
    # one scratch tensor per chunk so chunk-c readback only waits on
    # chunk-c's scatter; 256 cols so the XBAR transpose readback can use
    # 128-col tiles
    adN = [nc.dram_tensor(f"ad{c}", [128, 256], BF16) for c in range(4)]

    with ExitStack() as ctx:
        pw = ctx.enter_context(tc.tile_pool(name="weights", bufs=1))
        ps = ctx.enter_context(tc.tile_pool(name="seq", bufs=1))
        ph = ctx.enter_context(tc.tile_pool(name="heat_ps", bufs=1,
                                            space="PSUM"))

        # ---------------- prologue: parallel DMAs + PE heater ----------
        zz = ps.tile([128, 768], BF16, tag="zz")
        nc.vector.memset(zz[:], 0.0)

        Xp = ps.tile([96, 576], F32, tag="Xp")
        nc.gpsimd.memset(Xp[:, 0:32], 0.0)
        nc.gpsimd.memset(Xp[:, 544:576], 0.0)
        nc.sync.dma_start(Xp[:, 32:544], x_ap)

        shapes = tp['shapes']
        wb16a = pw.tile(list(tp['wb16a'][1]), BF16, tag="wb16a")
        nc.scalar.dma_start(wb16a[:], tp['wb16a'][0].ap())
        wf32 = pw.tile(list(tp['wf32'][1]), F32, tag="wf32")
        nc.scalar.dma_start(wf32[:], tp['wf32'][0].ap())
        wb16c = pw.tile(list(tp['wb16c'][1]), BF16, tag="wb16c")
        nc.scalar.dma_start(wb16c[:], tp['wb16c'][0].ap())

        w = {}
        for blob_tile, names in ((wf32, F32_PACK), (wb16a, B16A_PACK),
                                 (wb16c, B16C_PACK)):
            col = 0
            for n in names:
                r, cw = shapes[n]
                w[n] = blob_tile[0:r, col:col + cw]
                col += cw

        # preload exp_and_others table while ACT is idle (covers tanh+exp)
        dumt = ps.tile([1, 4], F32, tag="dumt")
        nc.scalar.activation(dumt[0:1, 0:1], zz[0:1, 0:1], AF.Tanh)

        # HAM warmup: back-to-back dummy matmuls on zeros
        hps = ph.tile([128, 512], F32, tag="heat")

        def heater(n):
            for _ in range(n):
                nc.tensor.matmul(hps[:], zz[:, 0:128], zz[:, 128:640])

        heater(N_HEAT_PROLOGUE)

        # zero adense scratch (sparsely written by the staircase DMA)
        for c in range(4):
            eng = nc.sync if c % 2 == 0 else nc.scalar
            eng.dma_start(bass.AP(adN[c], 0, [[256, 128], [1, 256]]),
                          zz[:, 0:256])

        # pads for conv stack tiles (gpsimd idle in prologue)
        h0 = ps.tile([96, 516], BF16, tag="h0")
        nc.gpsimd.memset(h0[:, 0:2], 0.0)
        nc.gpsimd.memset(h0[:, 514:516], 0.0)
        h1d = ps.tile([128, 1028], BF16, tag="h1d")
        nc.gpsimd.memset(h1d[:, 0:2], 0.0)
        nc.gpsimd.memset(h1d[:, 1024:1028], 0.0)
        h2d = ps.tile([112, 2052], BF16, tag="h2d")
        nc.gpsimd.memset(h2d[:, 0:2], 0.0)
        nc.gpsimd.memset(h2d[:, 2048:2052], 0.0)
        nc.gpsimd.memset(h2d[32:64, :], 0.0)
        h3rep = ps.tile([128, 4112], BF16, tag="h3rep")
        nc.gpsimd.memset(h3rep[0:32, 0:8], 0.0)
        nc.gpsimd.memset(h3rep[0:32, 4104:4112], 0.0)

        Xpb = ps.tile([96, 512], BF16, tag="Xpb")
        nc.vector.tensor_copy(Xpb[:], Xp[:, 32:544])

        # ---------------- attention: q/k ----------------
        Q4 = ps.tile([128, 128], BF16, tag="Q4")
        K4pad = ps.tile([128, 192], BF16, tag="K4pad")
        nc.gpsimd.memset(K4pad[0:32, 0:32], 0.0)
        nc.gpsimd.memset(K4pad[96:128, 160:192], 0.0)

        with tc.tile_pool(name="qk_ps", bufs=2, space="PSUM") as pp:
            k_ps = pp.tile([128, 128], F32, tag="qk")
            for c in range(4):
                nc.tensor.matmul(k_ps[32 * c:32 * c + 32, :], w['Wx'],
                                 Xpb[:, c * 128:(c + 1) * 128],
                                 tile_position=(0, 32 * c))
            nc.vector.tensor_scalar_add(K4pad[:, 32:160], k_ps[:],
                                        w['bh4col'])
            q_ps = pp.tile([128, 128], F32, tag="qk")
            for c in range(4):
                nc.tensor.matmul(q_ps[32 * c:32 * c + 32, :], w['Wt'],
                                 Xpb[:, c * 128:(c + 1) * 128],
                                 tile_position=(0, 32 * c))
            nc.vector.tensor_copy(Q4[:], q_ps[:])
        # cross-chunk halo wings (two HWDGE queues)
        nc.sync.dma_start(K4pad[32:128, 0:32], K4pad[0:96, 128:160])
        nc.sync.dma_start(K4pad[0:96, 160:192], K4pad[32:128, 32:64])
        # Q4-pinned heaters bridge the PE-idle window between the q/k
        # matmuls and the first band-logit burst (add0 + tanh0 latency)
        for _ in range(10):
            nc.tensor.matmul(hps[:], Q4[:], zz[:, 128:640])

        # X windows for AV: PE transposes emitted inside the band loop so
        # the scheduler runs them (and their DVE evacs) during the tanh
        # phase instead of ahead of the first Targ add
        Xw = []
        for c in range(4):
            xw = ps.tile([128, 192], BF16, tag=f"Xw{c}")
            Xw.append(xw)
        xt_pool = ctx.enter_context(tc.tile_pool(name="xw_ps", bufs=2,
                                                 space="PSUM"))

        def xw_transpose(c):
            x_ps = xt_pool.tile([128, 192], F32, tag="x")
            nc.tensor.transpose(x_ps[:, 0:96], Xp[:, c * 128:c * 128 + 128],
                                w['identity'][0:96, 0:96])
            nc.tensor.transpose(x_ps[0:64, 96:192],
                                Xp[:, c * 128 + 128:c * 128 + 192],
                                w['identity'][0:96, 0:96])
            nc.vector.tensor_copy(Xw[c][:, 0:96], x_ps[:, 0:96])
            nc.vector.tensor_copy(Xw[c][0:64, 96:192], x_ps[0:64, 96:192])

        # ---------------- attention: band logits ----------------
        GO = 16  # offsets per group
        Abf = ps.tile([128, 256], BF16, tag="Abf")
        S4 = ps.tile([128, 4], F32, tag="S4")

        with ExitStack() as ectx:
            pa_arg = ectx.enter_context(tc.tile_pool(name="arg_sb", bufs=2))
            pa_tan = ectx.enter_context(tc.tile_pool(name="tan_sb", bufs=3))
            pe = ectx.enter_context(tc.tile_pool(name="e_ps", bufs=1,
                                                 space="PSUM"))
            E_ps = pe.tile([128, 256], F32, tag="E")
            for g in range(64 // GO):
                o0 = GO * g
                Targ = pa_arg.tile([128, GO * 128], BF16, tag="Targ")
                q_b = _bcast_free(Q4[:], Q4[:], [[0, GO], [1, 128]])
                k_b = _bcast_free(K4pad[:], K4pad[:, o0:192], [[1, GO], [1, 128]])
                nc.vector.tensor_add(
                    Targ[:].rearrange("p (o i) -> p o i", o=GO), q_b, k_b)
                Ttan = pa_tan.tile([128, GO * 128], BF16, tag="Ttan")
                nc.scalar.activation(Ttan[:], Targ[:], AF.Tanh)
                for oi in range(GO):
                    o = o0 + oi
                    nc.tensor.matmul(
                        E_ps[:].rearrange("p (c o) -> p c o", o=64)[:, :, o],
                        Ttan[:, oi * 128:(oi + 1) * 128], w['blockWa4'])
                # dependency-pinned heater: keeps the HAM busy-window alive
                # during the ACT-bound tanh phase
                nc.tensor.matmul(hps[:], Ttan[:, 0:128], zz[:, 128:640])
                xw_transpose(g)
            EXb = ps.tile([128, 256], BF16, tag="EXb")
            nc.scalar.activation(EXb[:], E_ps[:], AF.Exp)

        # masked exp weights; band-row sums S; fold S into the center
        # column (o=32 <-> j=i) so AV yields u = sum_o e*x + S*x and the
        # softmax normalization cancels inside LN0 (scale invariance).
        nc.vector.tensor_mul(Abf[:], EXb[:], w['bmask16'])
        nc.vector.tensor_reduce(S4[:], Abf[:].rearrange("p (c o) -> p c o", o=64),
                                AX.X, ALU.add)
        ctr = Abf[:].rearrange("p (c o) -> p c o", o=64)[:, :, 32]
        nc.vector.tensor_add(ctr, ctr, S4[:])

        # preload sqrt_and_others table during the scatter/AV phase (the
        # dummy reads EXb so the scheduler keeps it after the attention exp)
        nc.scalar.activation(dumt[0:1, 1:2], EXb[0:1, 0:1], AF.Sqrt)

        # staircase scatter per chunk: adN[c][i][j'=i+o] = A[i, (c,o)]
        for c in range(4):
            eng = nc.sync if c % 2 == 0 else nc.scalar
            eng.dma_start(
                bass.AP(adN[c], 0, [[257, 128], [1, 64]]),
                Abf[:, c * 64:(c + 1) * 64])

        for _ in range(N_HEAT_SOFTMAX):
            nc.tensor.matmul(hps[:], EXb[:, 0:128], zz[:, 128:640])

        # ---------------- attention: AV (-> u, unnormalized) ------------
        u = ps.tile([96, 512], F32, tag="u")
        with ExitStack() as actx:
            pa = actx.enter_context(tc.tile_pool(name="av_sb", bufs=2))
            pt = actx.enter_context(tc.tile_pool(name="av_ps", bufs=2,
                                                 space="PSUM"))
            pv = actx.enter_context(tc.tile_pool(name="v_ps", bufs=2,
                                                 space="PSUM"))
            for c in range(4):
                Ad = pa.tile([128, 192], BF16, tag="Ad")
                eng = nc.scalar if c % 2 == 0 else nc.sync
                eng.dma_start(
                    Ad[:], bass.AP(adN[c], 0, [[256, 128], [1, 192]]))
                t_ps = pt.tile([128, 256], BF16, tag="tb")
                nc.tensor.transpose(t_ps[:, 0:128], Ad[:, 0:128], w['identb'])
                nc.tensor.transpose(t_ps[0:64, 128:256], Ad[:, 128:192],
                                    w['identb'])
                At = pa.tile([128, 256], BF16, tag="At")
                nc.vector.tensor_copy(At[:, 0:128], t_ps[:, 0:128])
                nc.vector.tensor_copy(At[0:64, 128:256], t_ps[0:64, 128:256])
                v_ps = pv.tile([96, 128], F32, tag="v")
                nc.tensor.matmul(v_ps[:], Xw[c][:, 0:96], At[:, 0:128],
                                 start=True, stop=False)
                nc.tensor.matmul(v_ps[:], Xw[c][0:64, 96:192], At[0:64, 128:256],
                                 start=False, stop=True)
                nc.vector.tensor_copy(u[:, c * 128:(c + 1) * 128], v_ps[:])

        # ---------------- LN / FFN ----------------
        def layer_norm(src_sb, grow, bcol, outs):
            # stats via 1.0-ones matmuls; sumsq matmul contracts an extra
            # 97th row holding 96*eps so var_raw = sum(x^2) + 96*eps - 96*mu^2
            # and rstd = recip(sqrt(var_raw/96)) needs no extra eps add.
            with ExitStack() as lctx:
                lp = lctx.enter_context(tc.tile_pool(name="ln_sb", bufs=1))
                lpp = lctx.enter_context(
                    tc.tile_pool(name="ln_ps", bufs=1, space="PSUM"))
                ub = lp.tile([96, 512], BF16, tag="ub")
                nc.vector.tensor_copy(ub[:], src_sb[:])
                sqb = lp.tile([97, 512], BF16, tag="sqb")
                nc.gpsimd.memset(sqb[96:97, :], 96.0 * EPS_LN)
                nc.vector.tensor_mul(sqb[0:96, :], ub[:], ub[:])
                mu_ps = lpp.tile([1, 512], F32, tag="lnmu")
                nc.tensor.matmul(mu_ps[:], w['onesc'][0:96, :], ub[:])
                q_ps = lpp.tile([1, 512], F32, tag="lnq")
                nc.tensor.matmul(q_ps[:], w['onesc'], sqb[:])
                mub = lp.tile([1, 512], BF16, tag="mub")
                nc.vector.tensor_scalar_mul(mub[:], mu_ps[:], 1.0 / 96.0)
                mub_ps = lpp.tile([96, 512], F32, tag="lnb")
                nc.tensor.matmul(mub_ps[:], w['one1b'], mub[:])
                m2 = lp.tile([1, 512], F32, tag="m2")
                nc.vector.tensor_mul(m2[:], mub[:], mub[:])
                var_r = lp.tile([1, 512], F32, tag="var")
                nc.vector.scalar_tensor_tensor(var_r[:], m2[:], -96.0,
                                               q_ps[:], ALU.mult, ALU.add)
                std = lp.tile([1, 512], F32, tag="std")
                nc.scalar.activation(std[:], var_r[:], AF.Sqrt,
                                     scale=1.0 / 96.0)
                rstdf = lp.tile([1, 512], F32, tag="rstdf")
                nc.vector.reciprocal_approx_fast(rstdf[:], std[:])
                rstdb = lp.tile([1, 512], BF16, tag="rstdb")
                nc.vector.tensor_copy(rstdb[:], rstdf[:])
                G_ps = lpp.tile([96, 512], F32, tag="lng")
                nc.tensor.matmul(G_ps[:], grow, rstdb[:])
                xc = lp.tile([96, 512], F32, tag="xc")
                nc.vector.tensor_sub(xc[:], src_sb[:], mub_ps[:])
                t2 = lp.tile([96, 512], F32, tag="t2")
                nc.vector.tensor_mul(t2[:], xc[:], G_ps[:])
                for out_ap in outs:
                    nc.vector.tensor_scalar_add(out_ap, t2[:], bcol[:])

        x2 = ps.tile([96, 512], F32, tag="x2")
        x2b = ps.tile([96, 512], BF16, tag="x2b")
        layer_norm(u, w['g0row'], w['b0col'], (x2[:], x2b[:]))

        x4pre = ps.tile([96, 512], F32, tag="x4pre")
        with ExitStack() as fctx:
            fp = fctx.enter_context(tc.tile_pool(name="ffn_sb", bufs=1))
            fpp = fctx.enter_context(tc.tile_pool(name="ffn_ps", bufs=1,
                                                  space="PSUM"))
            Hr = []
            for s in range(3):
                h_ps = fpp.tile([128, 512], F32, tag=f"h{s}")
                nc.tensor.matmul(h_ps[:], w['w0T'][:, s * 128:(s + 1) * 128],
                                 x2b[:])
                hr = fp.tile([128, 512], BF16, tag=f"hr{s}")
                # DVE relu: no ACT op here, so the sqrt table set stays
                # loaded between the two LayerNorms
                nc.vector.tensor_scalar(hr[:], h_ps[:],
                                        w['fb0'][:, s:s + 1], 0.0,
                                        ALU.add, ALU.max)
                Hr.append(hr)
            x3_ps = fpp.tile([96, 512], F32, tag="x3")
            for s in range(3):
                nc.tensor.matmul(x3_ps[:], w['w1T'][:, s * 96:(s + 1) * 96],
                                 Hr[s][:], start=(s == 0), stop=(s == 2))
            nc.vector.scalar_tensor_tensor(x4pre[:], x3_ps[:], w['fb1col'],
                                           x2[:], ALU.add, ALU.add)

        layer_norm(x4pre, w['g1row'], w['b1col'], (h0[:, 2:514],))

        # reload exp_and_others (relu+tanh) behind the conv1 matmuls; the
        # dummy reads h0 so it lands after LN1's sqrt on the ACT queue
        nc.scalar.activation(dumt[0:1, 2:3], h0[0:1, 2:3], AF.Relu)

        # ---------------- conv stack ----------------
        def evac(eng_act, dst, z_ps, bcol):
            if eng_act:
                nc.scalar.activation(dst, z_ps, AF.Relu, bias=bcol)
            else:
                nc.vector.tensor_scalar(dst, z_ps, bcol, 0.0, ALU.add, ALU.max)

        def strided_dst(hout, p0, cout, col0):
            return bass.AP(hout[:].tensor, hout[p0:p0 + 1, col0:col0 + 1].offset,
                           [[hout[:].ap[0][0], cout], [2, 512]])

        # conv1: ci=96 co=64, 4 taps, double-write into h1d (pack2 feed)
        with tc.tile_pool(name="c1_ps", bufs=2, space="PSUM") as cp:
            for par, Wp in ((0, w['W1e']), (1, w['W1o'])):
                z_ps = cp.tile([64, 512], F32, tag="z")
                for tau in range(4):
                    nc.tensor.matmul(z_ps[:], Wp[:, tau * 64:(tau + 1) * 64],
                                     h0[:, par + tau:par + tau + 512],
                                     start=(tau == 0), stop=(tau == 3))
                evac(par == 0, strided_dst(h1d, 0, 64, 2 + par), z_ps[:],
                     w['cb1'])
                evac(par == 1, strided_dst(h1d, 64, 64, 1 + par), z_ps[:],
                     w['cb1'])

        # conv2 (pack2): ci2=128 co=48, 2 tap-groups, double-write into h2d
        with tc.tile_pool(name="c2_ps", bufs=4, space="PSUM") as cp:
            for k in range(2):
                for par, Wp in ((0, w['W2e2']), (1, w['W2o2'])):
                    z_ps = cp.tile([48, 512], F32, tag="z")
                    for g in range(2):
                        nc.tensor.matmul(
                            z_ps[:], Wp[:, g * 48:(g + 1) * 48],
                            h1d[:, par + 2 * g + k * 512:
                                par + 2 * g + k * 512 + 512],
                            start=(g == 0), stop=(g == 1))
                    col0 = 2 * k * 512 + par
                    evac((k + par) % 2 == 0, strided_dst(h2d, 0, 48, col0 + 2),
                         z_ps[:], w['cb2'])
                    evac((k + par) % 2 == 1, strided_dst(h2d, 64, 48, col0 + 1),
                         z_ps[:], w['cb2'])

        # conv3 (pack2): ci2=112 (gapped) co=32, single write into h3rep[0:32]
        # tap replicas (rows 32r..32r+32 = h3 shifted left by r) pipeline
        # per output window, one window behind the evacs that feed them
        def replicate(k):
            a = 0 if k == 0 else 8 + k * 1024
            for r in range(1, 4):
                b = 4112 - r if k == 3 else 8 + (k + 1) * 1024
                eng = nc.sync if (3 * k + r) % 2 == 0 else nc.scalar
                eng.dma_start(h3rep[32 * r:32 * r + 32, a:b],
                              h3rep[0:32, a + r:b + r])

        with tc.tile_pool(name="c3_ps", bufs=4, space="PSUM") as cp:
            for k in range(4):
                for par, Wp in ((0, w['W3e2']), (1, w['W3o2'])):
                    z_ps = cp.tile([32, 512], F32, tag="z")
                    for g in range(2):
                        nc.tensor.matmul(
                            z_ps[:], Wp[:, g * 32:(g + 1) * 32],
                            h2d[:, par + 2 * g + k * 512:
                                par + 2 * g + k * 512 + 512],
                            start=(g == 0), stop=(g == 1))
                    evac((k + par) % 2 == 0,
                         strided_dst(h3rep, 0, 32, 8 + 2 * k * 512 + par),
                         z_ps[:], w['cb3'])
                if k > 0 and par == 1:
                    replicate(k - 1)
        replicate(3)

        # ---------------- output conv + sigmoid ----------------
        # 24 accumulating matmuls put chunk k's [1, 512] on partition k;
        # sigmoid(z+ob) = 0.5*tanh(0.5*z + ob/2) + 0.5 on 8 partitions.
        ysig = ps.tile([8, 512], F32, tag="ysig")
        with tc.tile_pool(name="yo_ps", bufs=1, space="PSUM") as yp:
            y_ps = yp.tile([8, 512], F32, tag="yo")
            for k in range(8):
                for g in range(3):
                    m = g * 8 + k
                    nc.tensor.matmul(
                        y_ps[:], w['Wog8'][:, m * 8:m * 8 + 8],
                        h3rep[:, k * 512 + 4 * g + 3:k * 512 + 4 * g + 3 + 512],
                        start=(k == 0 and g == 0), stop=(k == 7 and g == 2))
            t_sig = ps.tile([8, 512], F32, tag="t_sig")
            nc.scalar.activation(t_sig[:], y_ps[:], AF.Tanh, scale=0.5,
                                 bias=w['obh8'])
        nc.vector.tensor_scalar(ysig[:], t_sig[:], 0.5, 0.5,
                                ALU.mult, ALU.add)
        nc.sync.dma_start(bass.AP(t_out, 0, [[512, 8], [1, 512]]), ysig[:])


# ----------------------------------------------------------------------------
# public entry point
# ----------------------------------------------------------------------------

def build_module(p):
    nc = bacc.Bacc("TRN2", target_bir_lowering=False, debug=False)
    t_in = nc.dram_tensor("x", [C, L], F32, kind="ExternalInput")
    t_out = nc.dram_tensor("out", [1, 4096], F32, kind="ExternalOutput")
    tp = {}
    for blob, dt in (('wf32', F32), ('wb16a', BF16), ('wb16c', BF16)):
        tp[blob] = (nc.dram_tensor(blob, list(p[blob].shape), dt,
                                   kind="ExternalInput"), p[blob].shape, dt)
    tp['shapes'] = p['shapes']
    with tile.TileContext(nc) as tc:
        _build(nc, tc, t_in, t_out, tp)
    nc.compile()
    return nc


def kernel(**inputs):
    # The neuron compile cache keys on the HLO signature only (it does not
    # hash the embedded bass program), so a stale entry from a different
    # kernel revision with identical I/O shapes would silently load the
    # wrong NEFF. Purge unless the cache was stamped by this exact source.
    import hashlib
    import shutil
    me = hashlib.sha256(open(__file__, 'rb').read()).hexdigest()
    for cdir in ('/root/.neuron-compile-cache', '/var/tmp/neuron-compile-cache'):
        marker = os.path.join(cdir, '.kernel_src_hash')
        try:
            if open(marker).read() == me:
                continue
        except OSError:
            pass
        shutil.rmtree(cdir, ignore_errors=True)
        try:
            os.makedirs(cdir, exist_ok=True)
            with open(marker, 'w') as fh:
                fh.write(me)
        except OSError:
            pass

    x = np.asarray(inputs['x'], np.float32)          # (8, 96, 512)
    N = x.shape[0]
    p = _host_prep(inputs)
    nc = build_module(p)
    import ml_dtypes
    feed = {'wf32': np.ascontiguousarray(p['wf32'], np.float32),
            'wb16a': np.ascontiguousarray(p['wb16a'], ml_dtypes.bfloat16),
            'wb16c': np.ascontiguousarray(p['wb16c'], ml_dtypes.bfloat16)}
    in_maps = []
    for n in range(N):
        m = dict(feed)
        m['x'] = np.ascontiguousarray(x[n])
        in_maps.append(m)
    res = run_bass_kernel_spmd(nc, in_maps, core_ids=list(range(N)))
    global LAST_RESULTS
    LAST_RESULTS = res
    out = np.stack([res.results[n]['out'] for n in range(N)], axis=0)
    return out.astype(np.float32)


LAST_RESULTS = None


if __name__ == '__main__':
    print("kernel.py loaded OK")


# revision 57
# speedup vs baseline: 1.0273x; 1.0273x over previous
"""Trainium2 Bass kernel for nn_Decoder_16054587752897.

Decoder block: banded additive (Bahdanau) attention + LN + FFN + LN +
3x (nearest-upsample-2x + conv1d k=7 + relu) + conv1d k=11 + sigmoid.

Sharding: pure data parallel - batch N=8, one batch element per NeuronCore.

Key optimizations over the v1 kernel (112us):
 - PE HAM warmup: dummy matmul stream during the DMA prologue flips the
   clock gate to 8/8 (2.4 GHz) before real matmuls start; small heater
   bursts in long PE-idle phases prevent re-throttle.
 - Parallel prologue: input + 3 weight blobs on 4 different engine queues.
 - Softmax normalization dropped entirely: LayerNorm is invariant to a
   per-position positive scale, so the AV matmul computes
   u_i = sum_o e[i,o] x_j + S_i x_i (S folded into the band center
   column) and LN0(u) == LN0(v + x) to ~1e-7.
 - LN rstd via exp(-0.5*ln(var+eps)) so the whole kernel uses only two
   ACT table sets (exp_and_others -> natural_log_exp_and_others), one
   mid-kernel switch, both triggered early by dummy ops off the chain.
 - Final sigmoid via exp(-ln(1+exp(-x))) on the same table set, computed
   on an [8, 512] layout (output conv written to 8 partitions via
   per-chunk one-hot lhsT columns) instead of [1, 4096] on one lane.
 - conv2/conv3 tap-pair packing (pack2): 2 accumulating matmuls instead
   of 4 per output tile, fed by double-written (shifted) activations.
 - h3rep tap replicas via 3 parallel-queue DMAs instead of serial.
"""

import os
import sys

for _p in ("/opt/trn_rl_repo",):
    if _p not in sys.path:
        sys.path.insert(0, _p)

import math
import numpy as np
from contextlib import ExitStack

import concourse.bass as bass
import concourse.bacc as bacc
import concourse.mybir as mybir
import concourse.tile as tile
from concourse.bass_utils import run_bass_kernel_spmd

F32 = mybir.dt.float32
BF16 = mybir.dt.bfloat16
AF = mybir.ActivationFunctionType
ALU = mybir.AluOpType
AX = mybir.AxisListType

L = 512
C = 96
EPS_LN = 1e-5
LN96 = math.log(96.0)

N_HEAT_PROLOGUE = 12
N_HEAT_SOFTMAX = 6


# ----------------------------------------------------------------------------
# host-side constant prep (weight-only transforms)
# ----------------------------------------------------------------------------

def _host_prep(inp):
    f = lambda k: np.ascontiguousarray(np.asarray(inp[k], np.float32))
    p = {}
    p['Wt'] = f('Wt')                       # [96, 32] lhsT for q
    p['Wx'] = f('Wx')                       # [96, 32] lhsT for k
    Wa = f('Wa')[:, 0]
    blockWa4 = np.zeros((128, 4), np.float32)
    for c in range(4):
        blockWa4[32 * c:32 * c + 32, c] = Wa
    p['blockWa4'] = blockWa4
    p['bh4col'] = np.tile(f('bh'), 4).reshape(128, 1)
    il = np.arange(128)[:, None, None]
    cc = np.arange(4)[None, :, None]
    oo = np.arange(64)[None, None, :]
    jj = cc * 128 + il + oo - 32
    p['bmask16'] = ((jj >= 0) & (jj < L)).astype(np.float32).reshape(128, 256)
    p['identity'] = np.eye(128, dtype=np.float32)
    p['identb'] = np.eye(128, dtype=np.float32)
    p['onesb'] = np.ones((96, 1), np.float32)
    p['one1b'] = np.ones((1, 96), np.float32)
    p['g0row'] = f('ln0_g').reshape(1, 96)
    p['g1row'] = f('ln1_g').reshape(1, 96)
    p['b0col'] = f('ln0_b').reshape(96, 1)
    p['b1col'] = f('ln1_b').reshape(96, 1)
    p['w0T'] = np.ascontiguousarray(f('ff_w0').T)                # [96, 384]
    p['fb0'] = np.ascontiguousarray(f('ff_b0').reshape(3, 128).T)  # [128, 3]
    # w1T [128, 3*96]: cols s*96+c = ff_w1[c, s*128+h]
    w1 = f('ff_w1')                                              # [96, 384]
    w1T = np.zeros((128, 288), np.float32)
    for s in range(3):
        w1T[:, s * 96:(s + 1) * 96] = w1[:, s * 128:(s + 1) * 128].T
    p['w1T'] = w1T
    p['fb1col'] = f('ff_b1').reshape(96, 1)

    def eo(w):
        # w: [co, ci, 7] -> even/odd tap-summed lhsT banks [ci, 4*co]
        We = np.stack([w[:, :, 0], w[:, :, 1] + w[:, :, 2],
                       w[:, :, 3] + w[:, :, 4], w[:, :, 5] + w[:, :, 6]])
        Wo = np.stack([w[:, :, 0] + w[:, :, 1], w[:, :, 2] + w[:, :, 3],
                       w[:, :, 4] + w[:, :, 5], w[:, :, 6]])
        co, ci = w.shape[0], w.shape[1]
        pack = lambda Ws: np.ascontiguousarray(
            Ws.transpose(2, 0, 1).reshape(ci, 4 * co))
        return pack(We), pack(Wo)

    p['W1e'], p['W1o'] = eo(f('up_w0'))   # [96, 256]
    W2e, W2o = eo(f('up_w1'))             # [64, 192]
    W3e, W3o = eo(f('up_w2'))             # [48, 128]

    def pack2(W, ci, co):
        # W [ci, 4*co] tap-major -> [2*ci, 2*co]: rows tau*ci+c_i,
        # group g covers taps (2g, 2g+1)
        out = np.zeros((2 * ci, 2 * co), np.float32)
        for g in range(2):
            for tau in range(2):
                t = 2 * g + tau
                out[tau * ci:(tau + 1) * ci, g * co:(g + 1) * co] = \
                    W[:, t * co:(t + 1) * co]
        return out
    p['W2e2'] = pack2(W2e, 64, 48)   # [128, 96]
    p['W2o2'] = pack2(W2o, 64, 48)

    def pack2g(W, ci, co):
        # like pack2, but the tau=1 row block sits at partition 64 (engine
        # writes must start 32-aligned, so the shifted h2 copy lives at
        # rows 64:64+ci with zero-weight gap rows in between)
        out = np.zeros((64 + ci, 2 * co), np.float32)
        for g in range(2):
            for tau in range(2):
                t = 2 * g + tau
                out[tau * 64:tau * 64 + ci, g * co:(g + 1) * co] = \
                    W[:, t * co:(t + 1) * co]
        return out
    p['W3e2'] = pack2g(W3e, 48, 32)   # [112, 64]
    p['W3o2'] = pack2g(W3o, 48, 32)
    p['cb1'] = f('up_b0').reshape(64, 1)
    p['cb2'] = f('up_b1').reshape(48, 1)
    p['cb3'] = f('up_b2').reshape(32, 1)
    ow = f('out_w')[0]                    # (32, 11)
    # Wog8 [128, 192]: block m=(g*8+k) is [128, 8] with only col k nonzero
    # = tap-group-g column; accumulating all 24 into one [8, 512] PSUM bank
    # puts output chunk k on partition k.
    Wog8 = np.zeros((128, 192), np.float32)
    for g in range(3):
        col = np.zeros(128, np.float32)
        for r in range(4):
            t = 4 * g + r
            if t < 11:
                col[32 * r:32 * r + 32] = ow[:, t]
        for k in range(8):
            Wog8[:, (g * 8 + k) * 8 + k] = col
    p['Wog8'] = Wog8
    p['obh8'] = np.full((8, 1), f('out_b')[0] / 2.0, np.float32)
    p['onesc'] = np.ones((97, 1), np.float32)

    packed = {}
    for blob, names in (('wf32', F32_PACK), ('wb16a', B16A_PACK),
                        ('wb16c', B16C_PACK)):
        width = sum(p[n].shape[1] for n in names)
        buf = np.zeros((128, width), np.float32)
        col = 0
        for n in names:
            a = p[n]
            buf[:a.shape[0], col:col + a.shape[1]] = a
            col += a.shape[1]
        packed[blob] = buf
    shapes = {n: p[n].shape for n in F32_PACK + B16A_PACK + B16C_PACK}
    packed['shapes'] = shapes
    return packed


F32_PACK = ('identity', 'bh4col', 'b0col', 'b1col', 'fb0', 'fb1col',
            'cb1', 'cb2', 'cb3', 'obh8')
B16A_PACK = ('Wt', 'Wx', 'blockWa4', 'bmask16', 'one1b', 'onesc',
             'g0row', 'g1row', 'identb')
B16C_PACK = ('w0T', 'w1T', 'W1e', 'W1o', 'W2e2', 'W2o2', 'W3e2', 'W3o2',
             'Wog8')


# ----------------------------------------------------------------------------
# device kernel build
# ----------------------------------------------------------------------------

def _bcast_free(ap_full, offset_ap, counts):
    """Custom AP on the same tensor: dims [[pstep, 128]] + counts pairs."""
    pstep = ap_full.ap[0][0]
    return bass.AP(ap_full.tensor, offset_ap.offset,
                   [[pstep, ap_full.ap[0][1]]] + list(counts))


def _build(nc, tc, t_in, t_out, tp):
    x_ap = t_in.ap()          # [96, 512]
    # one scratch tensor per chunk so chunk-c readback only waits on
    # chunk-c's scatter; 256 cols so the XBAR transpose readback can use
    # 128-col tiles
    adN = [nc.dram_tensor(f"ad{c}", [128, 256], BF16) for c in range(4)]

    with ExitStack() as ctx:
        pw = ctx.enter_context(tc.tile_pool(name="weights", bufs=1))
        ps = ctx.enter_context(tc.tile_pool(name="seq", bufs=1))
        ph = ctx.enter_context(tc.tile_pool(name="heat_ps", bufs=1,
                                            space="PSUM"))

        # ---------------- prologue: parallel DMAs + PE heater ----------
        zz = ps.tile([128, 768], BF16, tag="zz")
        nc.vector.memset(zz[:], 0.0)

        Xp = ps.tile([96, 576], F32, tag="Xp")
        nc.gpsimd.memset(Xp[:, 0:32], 0.0)
        nc.gpsimd.memset(Xp[:, 544:576], 0.0)
        nc.sync.dma_start(Xp[:, 32:544], x_ap)

        shapes = tp['shapes']
        wb16a = pw.tile(list(tp['wb16a'][1]), BF16, tag="wb16a")
        nc.scalar.dma_start(wb16a[:], tp['wb16a'][0].ap())
        wf32 = pw.tile(list(tp['wf32'][1]), F32, tag="wf32")
        nc.scalar.dma_start(wf32[:], tp['wf32'][0].ap())
        wb16c = pw.tile(list(tp['wb16c'][1]), BF16, tag="wb16c")
        nc.scalar.dma_start(wb16c[:], tp['wb16c'][0].ap())

        w = {}
        for blob_tile, names in ((wf32, F32_PACK), (wb16a, B16A_PACK),
                                 (wb16c, B16C_PACK)):
            col = 0
            for n in names:
                r, cw = shapes[n]
                w[n] = blob_tile[0:r, col:col + cw]
                col += cw

        # preload exp_and_others table while ACT is idle (covers tanh+exp)
        dumt = ps.tile([1, 4], F32, tag="dumt")
        nc.scalar.activation(dumt[0:1, 0:1], zz[0:1, 0:1], AF.Tanh)

        # HAM warmup: back-to-back dummy matmuls on zeros
        hps = ph.tile([128, 512], F32, tag="heat")

        def heater(n):
            for _ in range(n):
                nc.tensor.matmul(hps[:], zz[:, 0:128], zz[:, 128:640])

        heater(N_HEAT_PROLOGUE)

        # zero adense scratch (sparsely written by the staircase DMA)
        for c in range(4):
            eng = nc.sync if c % 2 == 0 else nc.scalar
            eng.dma_start(bass.AP(adN[c], 0, [[256, 128], [1, 256]]),
                          zz[:, 0:256])

        # pads for conv stack tiles (gpsimd idle in prologue)
        h0 = ps.tile([96, 516], BF16, tag="h0")
        nc.gpsimd.memset(h0[:, 0:2], 0.0)
        nc.gpsimd.memset(h0[:, 514:516], 0.0)
        h1d = ps.tile([128, 1028], BF16, tag="h1d")
        nc.gpsimd.memset(h1d[:, 0:2], 0.0)
        nc.gpsimd.memset(h1d[:, 1024:1028], 0.0)
        h2d = ps.tile([112, 2052], BF16, tag="h2d")
        nc.gpsimd.memset(h2d[:, 0:2], 0.0)
        nc.gpsimd.memset(h2d[:, 2048:2052], 0.0)
        nc.gpsimd.memset(h2d[32:64, :], 0.0)
        h3rep = ps.tile([128, 4112], BF16, tag="h3rep")
        nc.gpsimd.memset(h3rep[0:32, 0:8], 0.0)
        nc.gpsimd.memset(h3rep[0:32, 4104:4112], 0.0)

        Xpb = ps.tile([96, 512], BF16, tag="Xpb")
        nc.vector.tensor_copy(Xpb[:], Xp[:, 32:544])

        # ---------------- attention: q/k ----------------
        Q4 = ps.tile([128, 128], BF16, tag="Q4")
        K4pad = ps.tile([128, 192], BF16, tag="K4pad")
        nc.gpsimd.memset(K4pad[0:32, 0:32], 0.0)
        nc.gpsimd.memset(K4pad[96:128, 160:192], 0.0)

        with tc.tile_pool(name="qk_ps", bufs=2, space="PSUM") as pp:
            k_ps = pp.tile([128, 128], F32, tag="qk")
            for c in range(4):
                nc.tensor.matmul(k_ps[32 * c:32 * c + 32, :], w['Wx'],
                                 Xpb[:, c * 128:(c + 1) * 128],
                                 tile_position=(0, 32 * c))
            nc.vector.tensor_scalar_add(K4pad[:, 32:160], k_ps[:],
                                        w['bh4col'])
            q_ps = pp.tile([128, 128], F32, tag="qk")
            for c in range(4):
                nc.tensor.matmul(q_ps[32 * c:32 * c + 32, :], w['Wt'],
                                 Xpb[:, c * 128:(c + 1) * 128],
                                 tile_position=(0, 32 * c))
            nc.vector.tensor_copy(Q4[:], q_ps[:])
        # cross-chunk halo wings (two HWDGE queues)
        nc.sync.dma_start(K4pad[32:128, 0:32], K4pad[0:96, 128:160])
        nc.sync.dma_start(K4pad[0:96, 160:192], K4pad[32:128, 32:64])
        # Q4-pinned heaters bridge the PE-idle window between the q/k
        # matmuls and the first band-logit burst (add0 + tanh0 latency)
        for _ in range(10):
            nc.tensor.matmul(hps[:], Q4[:], zz[:, 128:640])

        # X windows for AV: PE transposes emitted inside the band loop so
        # the scheduler runs them (and their DVE evacs) during the tanh
        # phase instead of ahead of the first Targ add
        Xw = []
        for c in range(4):
            xw = ps.tile([128, 192], BF16, tag=f"Xw{c}")
            Xw.append(xw)
        xt_pool = ctx.enter_context(tc.tile_pool(name="xw_ps", bufs=2,
                                                 space="PSUM"))

        def xw_transpose(c):
            x_ps = xt_pool.tile([128, 192], F32, tag="x")
            nc.tensor.transpose(x_ps[:, 0:96], Xp[:, c * 128:c * 128 + 128],
                                w['identity'][0:96, 0:96])
            nc.tensor.transpose(x_ps[0:64, 96:192],
                                Xp[:, c * 128 + 128:c * 128 + 192],
                                w['identity'][0:96, 0:96])
            nc.vector.tensor_copy(Xw[c][:, 0:96], x_ps[:, 0:96])
            nc.vector.tensor_copy(Xw[c][0:64, 96:192], x_ps[0:64, 96:192])

        # ---------------- attention: band logits ----------------
        GO = 16  # offsets per group
        Abf = ps.tile([128, 256], BF16, tag="Abf")
        S4 = ps.tile([128, 4], F32, tag="S4")

        with ExitStack() as ectx:
            pa_arg = ectx.enter_context(tc.tile_pool(name="arg_sb", bufs=2))
            pa_tan = ectx.enter_context(tc.tile_pool(name="tan_sb", bufs=3))
            pe = ectx.enter_context(tc.tile_pool(name="e_ps", bufs=1,
                                                 space="PSUM"))
            E_ps = pe.tile([128, 256], F32, tag="E")
            for g in range(64 // GO):
                o0 = GO * g
                Targ = pa_arg.tile([128, GO * 128], BF16, tag="Targ")
                q_b = _bcast_free(Q4[:], Q4[:], [[0, GO], [1, 128]])
                k_b = _bcast_free(K4pad[:], K4pad[:, o0:192], [[1, GO], [1, 128]])
                nc.vector.tensor_add(
                    Targ[:].rearrange("p (o i) -> p o i", o=GO), q_b, k_b)
                Ttan = pa_tan.tile([128, GO * 128], BF16, tag="Ttan")
                nc.scalar.activation(Ttan[:], Targ[:], AF.Tanh)
                for oi in range(GO):
                    o = o0 + oi
                    nc.tensor.matmul(
                        E_ps[:].rearrange("p (c o) -> p c o", o=64)[:, :, o],
                        Ttan[:, oi * 128:(oi + 1) * 128], w['blockWa4'])
                # X-window transposes double as PE keep-warm filler during
                # the ACT-bound tanh phase
                xw_transpose(g)
            EXb = ps.tile([128, 256], BF16, tag="EXb")
            nc.scalar.activation(EXb[:], E_ps[:], AF.Exp)

        # masked exp weights; band-row sums S; fold S into the center
        # column (o=32 <-> j=i) so AV yields u = sum_o e*x + S*x and the
        # softmax normalization cancels inside LN0 (scale invariance).
        nc.vector.tensor_mul(Abf[:], EXb[:], w['bmask16'])
        nc.vector.tensor_reduce(S4[:], Abf[:].rearrange("p (c o) -> p c o", o=64),
                                AX.X, ALU.add)
        ctr = Abf[:].rearrange("p (c o) -> p c o", o=64)[:, :, 32]
        nc.vector.tensor_add(ctr, ctr, S4[:])

        # preload sqrt_and_others table during the scatter/AV phase (the
        # dummy reads EXb so the scheduler keeps it after the attention exp)
        nc.scalar.activation(dumt[0:1, 1:2], EXb[0:1, 0:1], AF.Sqrt)

        # staircase scatter per chunk: adN[c][i][j'=i+o] = A[i, (c,o)]
        for c in range(4):
            eng = nc.sync if c % 2 == 0 else nc.scalar
            eng.dma_start(
                bass.AP(adN[c], 0, [[257, 128], [1, 64]]),
                Abf[:, c * 64:(c + 1) * 64])

        for _ in range(N_HEAT_SOFTMAX):
            nc.tensor.matmul(hps[:], EXb[:, 0:128], zz[:, 128:640])

        # ---------------- attention: AV (-> u, unnormalized) ------------
        u = ps.tile([96, 512], F32, tag="u")
        with ExitStack() as actx:
            pa = actx.enter_context(tc.tile_pool(name="av_sb", bufs=2))
            pt = actx.enter_context(tc.tile_pool(name="av_ps", bufs=2,
                                                 space="PSUM"))
            pv = actx.enter_context(tc.tile_pool(name="v_ps", bufs=2,
                                                 space="PSUM"))
            for c in range(4):
                Ad = pa.tile([128, 192], BF16, tag="Ad")
                eng = nc.scalar if c % 2 == 0 else nc.sync
                eng.dma_start(
                    Ad[:], bass.AP(adN[c], 0, [[256, 128], [1, 192]]))
                t_ps = pt.tile([128, 256], BF16, tag="tb")
                nc.tensor.transpose(t_ps[:, 0:128], Ad[:, 0:128], w['identb'])
                nc.tensor.transpose(t_ps[0:64, 128:256], Ad[:, 128:192],
                                    w['identb'])
                At = pa.tile([128, 256], BF16, tag="At")
                nc.vector.tensor_copy(At[:, 0:128], t_ps[:, 0:128])
                nc.vector.tensor_copy(At[0:64, 128:256], t_ps[0:64, 128:256])
                v_ps = pv.tile([96, 128], F32, tag="v")
                nc.tensor.matmul(v_ps[:], Xw[c][:, 0:96], At[:, 0:128],
                                 start=True, stop=False)
                nc.tensor.matmul(v_ps[:], Xw[c][0:64, 96:192], At[0:64, 128:256],
                                 start=False, stop=True)
                nc.vector.tensor_copy(u[:, c * 128:(c + 1) * 128], v_ps[:])

        # ---------------- LN / FFN ----------------
        def layer_norm(src_sb, grow, bcol, outs):
            # stats via 1.0-ones matmuls; sumsq matmul contracts an extra
            # 97th row holding 96*eps so var_raw = sum(x^2) + 96*eps - 96*mu^2
            # and rstd = recip(sqrt(var_raw/96)) needs no extra eps add.
            with ExitStack() as lctx:
                lp = lctx.enter_context(tc.tile_pool(name="ln_sb", bufs=1))
                lpp = lctx.enter_context(
                    tc.tile_pool(name="ln_ps", bufs=1, space="PSUM"))
                ub = lp.tile([96, 512], BF16, tag="ub")
                nc.vector.tensor_copy(ub[:], src_sb[:])
                sqb = lp.tile([97, 512], BF16, tag="sqb")
                nc.gpsimd.memset(sqb[96:97, :], 96.0 * EPS_LN)
                nc.vector.tensor_mul(sqb[0:96, :], ub[:], ub[:])
                mu_ps = lpp.tile([1, 512], F32, tag="lnmu")
                nc.tensor.matmul(mu_ps[:], w['onesc'][0:96, :], ub[:])
                q_ps = lpp.tile([1, 512], F32, tag="lnq")
                nc.tensor.matmul(q_ps[:], w['onesc'], sqb[:])
                mub = lp.tile([1, 512], BF16, tag="mub")
                nc.vector.tensor_scalar_mul(mub[:], mu_ps[:], 1.0 / 96.0)
                mub_ps = lpp.tile([96, 512], F32, tag="lnb")
                nc.tensor.matmul(mub_ps[:], w['one1b'], mub[:])
                m2 = lp.tile([1, 512], F32, tag="m2")
                nc.vector.tensor_mul(m2[:], mub[:], mub[:])
                var_r = lp.tile([1, 512], F32, tag="var")
                nc.vector.scalar_tensor_tensor(var_r[:], m2[:], -96.0,
                                               q_ps[:], ALU.mult, ALU.add)
                std = lp.tile([1, 512], F32, tag="std")
                nc.scalar.activation(std[:], var_r[:], AF.Sqrt,
                                     scale=1.0 / 96.0)
                rstdf = lp.tile([1, 512], F32, tag="rstdf")
                nc.vector.reciprocal_approx_fast(rstdf[:], std[:])
                rstdb = lp.tile([1, 512], BF16, tag="rstdb")
                nc.vector.tensor_copy(rstdb[:], rstdf[:])
                G_ps = lpp.tile([96, 512], F32, tag="lng")
                nc.tensor.matmul(G_ps[:], grow, rstdb[:])
                xc = lp.tile([96, 512], F32, tag="xc")
                nc.vector.tensor_sub(xc[:], src_sb[:], mub_ps[:])
                t2 = lp.tile([96, 512], F32, tag="t2")
                nc.vector.tensor_mul(t2[:], xc[:], G_ps[:])
                for out_ap in outs:
                    nc.vector.tensor_scalar_add(out_ap, t2[:], bcol[:])

        x2 = ps.tile([96, 512], F32, tag="x2")
        x2b = ps.tile([96, 512], BF16, tag="x2b")
        layer_norm(u, w['g0row'], w['b0col'], (x2[:], x2b[:]))

        x4pre = ps.tile([96, 512], F32, tag="x4pre")
        with ExitStack() as fctx:
            fp = fctx.enter_context(tc.tile_pool(name="ffn_sb", bufs=1))
            fpp = fctx.enter_context(tc.tile_pool(name="ffn_ps", bufs=1,
                                                  space="PSUM"))
            Hr = []
            for s in range(3):
                h_ps = fpp.tile([128, 512], F32, tag=f"h{s}")
                nc.tensor.matmul(h_ps[:], w['w0T'][:, s * 128:(s + 1) * 128],
                                 x2b[:])
                hr = fp.tile([128, 512], BF16, tag=f"hr{s}")
                # DVE relu: no ACT op here, so the sqrt table set stays
                # loaded between the two LayerNorms
                nc.vector.tensor_scalar(hr[:], h_ps[:],
                                        w['fb0'][:, s:s + 1], 0.0,
                                        ALU.add, ALU.max)
                Hr.append(hr)
            x3_ps = fpp.tile([96, 512], F32, tag="x3")
            for s in range(3):
                nc.tensor.matmul(x3_ps[:], w['w1T'][:, s * 96:(s + 1) * 96],
                                 Hr[s][:], start=(s == 0), stop=(s == 2))
            heater(4)
            nc.vector.scalar_tensor_tensor(x4pre[:], x3_ps[:], w['fb1col'],
                                           x2[:], ALU.add, ALU.add)

        layer_norm(x4pre, w['g1row'], w['b1col'], (h0[:, 2:514],))

        # reload exp_and_others (relu+tanh) behind the conv1 matmuls; the
        # dummy reads h0 so it lands after LN1's sqrt on the ACT queue
        nc.scalar.activation(dumt[0:1, 2:3], h0[0:1, 2:3], AF.Relu)

        # ---------------- conv stack ----------------
        def evac(eng_act, dst, z_ps, bcol):
            if eng_act:
                nc.scalar.activation(dst, z_ps, AF.Relu, bias=bcol)
            else:
                nc.vector.tensor_scalar(dst, z_ps, bcol, 0.0, ALU.add, ALU.max)

        def strided_dst(hout, p0, cout, col0):
            return bass.AP(hout[:].tensor, hout[p0:p0 + 1, col0:col0 + 1].offset,
                           [[hout[:].ap[0][0], cout], [2, 512]])

        # conv1: ci=96 co=64, 4 taps, double-write into h1d (pack2 feed)
        with tc.tile_pool(name="c1_ps", bufs=2, space="PSUM") as cp:
            for par, Wp in ((0, w['W1e']), (1, w['W1o'])):
                z_ps = cp.tile([64, 512], F32, tag="z")
                for tau in range(4):
                    nc.tensor.matmul(z_ps[:], Wp[:, tau * 64:(tau + 1) * 64],
                                     h0[:, par + tau:par + tau + 512],
                                     start=(tau == 0), stop=(tau == 3))
                evac(par == 0, strided_dst(h1d, 0, 64, 2 + par), z_ps[:],
                     w['cb1'])
                evac(par == 1, strided_dst(h1d, 64, 64, 1 + par), z_ps[:],
                     w['cb1'])
            heater(5)

        # conv2 (pack2): ci2=128 co=48, 2 tap-groups, double-write into h2d
        with tc.tile_pool(name="c2_ps", bufs=4, space="PSUM") as cp:
            for k in range(2):
                for par, Wp in ((0, w['W2e2']), (1, w['W2o2'])):
                    z_ps = cp.tile([48, 512], F32, tag="z")
                    for g in range(2):
                        nc.tensor.matmul(
                            z_ps[:], Wp[:, g * 48:(g + 1) * 48],
                            h1d[:, par + 2 * g + k * 512:
                                par + 2 * g + k * 512 + 512],
                            start=(g == 0), stop=(g == 1))
                    col0 = 2 * k * 512 + par
                    evac((k + par) % 2 == 0, strided_dst(h2d, 0, 48, col0 + 2),
                         z_ps[:], w['cb2'])
                    evac((k + par) % 2 == 1, strided_dst(h2d, 64, 48, col0 + 1),
                         z_ps[:], w['cb2'])
            heater(5)

        # conv3 (pack2): ci2=112 (gapped) co=32, single write into h3rep[0:32]
        # tap replicas (rows 32r..32r+32 = h3 shifted left by r) pipeline
        # per output window, one window behind the evacs that feed them
        def replicate(k):
            a = 0 if k == 0 else 8 + k * 1024
            for r in range(1, 4):
                b = 4112 - r if k == 3 else 8 + (k + 1) * 1024
                eng = nc.sync if (3 * k + r) % 2 == 0 else nc.scalar
                eng.dma_start(h3rep[32 * r:32 * r + 32, a:b],
                              h3rep[0:32, a + r:b + r])

        with tc.tile_pool(name="c3_ps", bufs=4, space="PSUM") as cp:
            for k in range(4):
                for par, Wp in ((0, w['W3e2']), (1, w['W3o2'])):
                    z_ps = cp.tile([32, 512], F32, tag="z")
                    for g in range(2):
                        nc.tensor.matmul(
                            z_ps[:], Wp[:, g * 32:(g + 1) * 32],
                            h2d[:, par + 2 * g + k * 512:
                                par + 2 * g + k * 512 + 512],
                            start=(g == 0), stop=(g == 1))
                    evac((k + par) % 2 == 0,
                         strided_dst(h3rep, 0, 32, 8 + 2 * k * 512 + par),
                         z_ps[:], w['cb3'])
                if k > 0 and par == 1:
                    replicate(k - 1)
            heater(2)
        replicate(3)

        # ---------------- output conv + sigmoid ----------------
        # 24 accumulating matmuls put chunk k's [1, 512] on partition k;
        # sigmoid(z+ob) = 0.5*tanh(0.5*z + ob/2) + 0.5 on 8 partitions.
        ysig = ps.tile([8, 512], F32, tag="ysig")
        with tc.tile_pool(name="yo_ps", bufs=1, space="PSUM") as yp:
            y_ps = yp.tile([8, 512], F32, tag="yo")
            for k in range(8):
                for g in range(3):
                    m = g * 8 + k
                    nc.tensor.matmul(
                        y_ps[:], w['Wog8'][:, m * 8:m * 8 + 8],
                        h3rep[:, k * 512 + 4 * g + 3:k * 512 + 4 * g + 3 + 512],
                        start=(k == 0 and g == 0), stop=(k == 7 and g == 2))
            t_sig = ps.tile([8, 512], F32, tag="t_sig")
            nc.scalar.activation(t_sig[:], y_ps[:], AF.Tanh, scale=0.5,
                                 bias=w['obh8'])
        nc.vector.tensor_scalar(ysig[:], t_sig[:], 0.5, 0.5,
                                ALU.mult, ALU.add)
        nc.sync.dma_start(bass.AP(t_out, 0, [[512, 8], [1, 512]]), ysig[:])


# ----------------------------------------------------------------------------
# public entry point
# ----------------------------------------------------------------------------

def build_module(p):
    nc = bacc.Bacc("TRN2", target_bir_lowering=False, debug=False)
    t_in = nc.dram_tensor("x", [C, L], F32, kind="ExternalInput")
    t_out = nc.dram_tensor("out", [1, 4096], F32, kind="ExternalOutput")
    tp = {}
    for blob, dt in (('wf32', F32), ('wb16a', BF16), ('wb16c', BF16)):
        tp[blob] = (nc.dram_tensor(blob, list(p[blob].shape), dt,
                                   kind="ExternalInput"), p[blob].shape, dt)
    tp['shapes'] = p['shapes']
    with tile.TileContext(nc) as tc:
        _build(nc, tc, t_in, t_out, tp)
    nc.compile()
    return nc


def kernel(**inputs):
    # The neuron compile cache keys on the HLO signature only (it does not
    # hash the embedded bass program), so a stale entry from a different
    # kernel revision with identical I/O shapes would silently load the
    # wrong NEFF. Purge unless the cache was stamped by this exact source.
    import hashlib
    import shutil
    me = hashlib.sha256(open(__file__, 'rb').read()).hexdigest()
    for cdir in ('/root/.neuron-compile-cache', '/var/tmp/neuron-compile-cache'):
        marker = os.path.join(cdir, '.kernel_src_hash')
        try:
            if open(marker).read() == me:
                continue
        except OSError:
            pass
        shutil.rmtree(cdir, ignore_errors=True)
        try:
            os.makedirs(cdir, exist_ok=True)
            with open(marker, 'w') as fh:
                fh.write(me)
        except OSError:
            pass

    x = np.asarray(inputs['x'], np.float32)          # (8, 96, 512)
    N = x.shape[0]
    p = _host_prep(inputs)
    nc = build_module(p)
    import ml_dtypes
    feed = {'wf32': np.ascontiguousarray(p['wf32'], np.float32),
            'wb16a': np.ascontiguousarray(p['wb16a'], ml_dtypes.bfloat16),
            'wb16c': np.ascontiguousarray(p['wb16c'], ml_dtypes.bfloat16)}
    in_maps = []
    for n in range(N):
        m = dict(feed)
        m['x'] = np.ascontiguousarray(x[n])
        in_maps.append(m)
    res = run_bass_kernel_spmd(nc, in_maps, core_ids=list(range(N)))
    global LAST_RESULTS
    LAST_RESULTS = res
    out = np.stack([res.results[n]['out'] for n in range(N)], axis=0)
    return out.astype(np.float32)


LAST_RESULTS = None


if __name__ == '__main__':
    print("kernel.py loaded OK")
